# revision 1
# baseline (speedup 1.0000x reference)
"""CenterLoss kernel for Trainium2 (raw Bass/Bacc, no Tile), 8-core
data-parallel.

Key algebraic insight: the reference builds the full [B, C] squared-
distance matrix and masks it with one-hot(labels), so only
distmat[i, labels[i]] survives.  The loss is therefore

    loss = (1/B) * sum_i || x_i - centers[labels[i]] ||^2

which needs only a gather of each sample's center row (indirect DMA), not
the [4096, 10000] matmul.

Sharding: data-parallel over the batch.  Each of the 8 cores gets 512
samples (x shard + labels shard) and the full replicated centers table in
DRAM; it gathers its 512 center rows, computes
(sum ||x - c||^2) / B on device, and the host all-reduces (sums) the 8
partial scalars.

Per core (512 samples = 4 chunks x 128 partitions, interleaved layout:
chunk a holds samples {4p + a}, one per partition p):
  Sync   : labels DMA ([128,4] int32 tile, 16 B strips), then x as two
           DMAs with 4 KB contiguous per-partition strips, out DMA
  GpSimd : ones memset, 4 indirect gathers (offset AP = labels column a,
           one index per partition; one DMA sem lane per gather)
  Vector : per chunk subtract
  Scalar : per chunk Square activation w/ accum; final PSUM->SBUF copy
  Tensor : partition reduction accumulated in PSUM: one [1,1] matmul
           partials[:,a].T @ (ones/B) per chunk as its accum lands

Manual semaphores; no Tile exit drain+butterfly+sem-clear (the bass entry
preamble clears sems, so re-execution stays safe).
"""

from contextlib import ExitStack

import numpy as np

import concourse.bacc as bacc
import concourse.bass as bass
from concourse import mybir
from concourse.bass_utils import run_bass_kernel_spmd

BATCH = 4096
NUM_CLASSES = 10000
FEAT_DIM = 512
N_CORES = 8
BPC = BATCH // N_CORES   # samples per core = 512
P = 128                  # SBUF partitions
CHUNKS = BPC // P        # 4 chunks of 128 samples per core

AF = mybir.AluOpType

_NC_CACHE = {}


def _build_bass():
    nc = bacc.Bacc(None, target_bir_lowering=False)

    x_in = nc.dram_tensor("x", [BPC, FEAT_DIM], mybir.dt.float32,
                          kind="ExternalInput")
    lab_in = nc.dram_tensor("labels", [BPC], mybir.dt.int32,
                            kind="ExternalInput")
    cen_in = nc.dram_tensor("centers", [NUM_CLASSES, FEAT_DIM],
                            mybir.dt.float32, kind="ExternalInput")
    out_t = nc.dram_tensor("out", [1, 1], mybir.dt.float32,
                           kind="ExternalOutput")

    with ExitStack() as ctx:
        ec = ctx.enter_context
        lab_sb = ec(nc.sbuf_tensor("lab_sb", [P, CHUNKS], mybir.dt.int32))
        xt = ec(nc.sbuf_tensor("xt", [P, CHUNKS * FEAT_DIM],
                               mybir.dt.float32))
        ct = ec(nc.sbuf_tensor("ct", [P, CHUNKS * FEAT_DIM],
                               mybir.dt.float32))
        dds = [ec(nc.sbuf_tensor(f"dd{a}", [P, FEAT_DIM], mybir.dt.float32))
               for a in range(CHUNKS)]
        sqs = [ec(nc.sbuf_tensor(f"sq{a}", [P, FEAT_DIM], mybir.dt.float32))
               for a in range(CHUNKS)]
        partials = ec(nc.sbuf_tensor("partials", [P, CHUNKS],
                                     mybir.dt.float32))
        ones = ec(nc.sbuf_tensor("ones", [P, 1], mybir.dt.float32))
        res = ec(nc.sbuf_tensor("res", [1, 1], mybir.dt.float32))
        ps = ec(nc.psum_tensor("ps", [1, 1], mybir.dt.float32))
        s_lab = ec(nc.semaphore("s_lab"))
        s_xs = [ec(nc.semaphore(f"s_x{a}")) for a in range(CHUNKS)]
        s_cts = [ec(nc.semaphore(f"s_ct{a}")) for a in range(CHUNKS)]
        s_g = ec(nc.semaphore("s_g"))
        s_sub = ec(nc.semaphore("s_sub"))
        s_acc = ec(nc.semaphore("s_acc"))
        s_pe = ec(nc.semaphore("s_pe"))
        s_res = ec(nc.semaphore("s_res"))
        s_out = ec(nc.semaphore("s_out"))

        # ---- Sync: labels first (gathers depend on them), then x as two
        # halves with 4 KB contiguous strips (partition p holds rows
        # 4p..4p+3; half h covers chunks {2h, 2h+1} = rows 4p+2h, 4p+2h+1).
        nc.sync.dma_start(
            out=lab_sb[:],
            in_=lab_in[:].rearrange("(p a) -> p a", a=CHUNKS),
        ).then_inc(s_lab, 16)
        H = CHUNKS // 2
        for h in range(2):
            nc.sync.dma_start(
                out=xt[:, h * H * FEAT_DIM:(h + 1) * H * FEAT_DIM],
                in_=x_in[:].rearrange(
                    "(p h g) f -> p h (g f)", h=2, g=H)[:, h, :],
            ).then_inc(s_xs[h], 16)

        # ---- GpSimd: ones + gathers (SWDGE) ----
        nc.gpsimd.memset(ones[:], 1.0 / BATCH).then_inc(s_g, 1)
        nc.gpsimd.wait_ge(s_lab, 16)
        for a in range(CHUNKS):
            nc.gpsimd.indirect_dma_start(
                out=ct[:, a * FEAT_DIM:(a + 1) * FEAT_DIM],
                out_offset=None,
                in_=cen_in[:],
                in_offset=bass.IndirectOffsetOnAxis(
                    ap=lab_sb[:, a:a + 1], axis=0),
            ).then_inc(s_cts[a], 16)

        # ---- Vector: per-chunk subtract ----
        for a in range(CHUNKS):
            sl = slice(a * FEAT_DIM, (a + 1) * FEAT_DIM)
            nc.vector.wait_ge(s_xs[a // (CHUNKS // 2)], 16)
            nc.vector.wait_ge(s_cts[a], 16)
            nc.vector.tensor_tensor(
                out=dds[a][:], in0=xt[:, sl], in1=ct[:, sl],
                op=AF.subtract).then_inc(s_sub, 1)

        # ---- Scalar: per-chunk square + accumulate along free dim ----
        for a in range(CHUNKS):
            nc.scalar.wait_ge(s_sub, a + 1)
            nc.scalar.activation(
                out=sqs[a][:], in_=dds[a][:],
                func=mybir.ActivationFunctionType.Square,
                accum_out=partials[:, a:a + 1]).then_inc(s_acc, 1)

        # ---- Tensor: partition reduction accumulated in PSUM, one
        # matmul per chunk as soon as its ACT accum column lands (PE works
        # inside the gather window; no separate free-dim reduce needed) ----
        nc.tensor.wait_ge(s_g, 1)
        for a in range(CHUNKS):
            nc.tensor.wait_ge(s_acc, a + 1)
            nc.tensor.matmul(out=ps[:], lhsT=partials[:, a:a + 1],
                             rhs=ones[:], start=(a == 0),
                             stop=(a == CHUNKS - 1)).then_inc(s_pe, 1)

        # ---- Scalar: PSUM -> SBUF ----
        nc.scalar.wait_ge(s_pe, CHUNKS)
        nc.scalar.copy(res[:], ps[:]).then_inc(s_res, 1)

        # ---- Sync: output DMA.  No completion wait: the NRT exit
        # barrier's per-engine Drain empties Sync's HWDGE queue before
        # execution is reported complete, so the data is guaranteed
        # written; halting at issue starts the exit rounds earlier. ----
        nc.sync.wait_ge(s_res, 1)
        nc.sync.dma_start(out=out_t[:], in_=res[:]).then_inc(s_out, 16)

    # Bacc defers register allocation + event-semaphore splitting to
    # compile(); the pjrt exec path serializes without calling it.
    nc.compile()
    return nc


def get_nc():
    if "nc" not in _NC_CACHE:
        _NC_CACHE["nc"] = _build_bass()
    return _NC_CACHE["nc"]


def kernel(x, labels, centers, _run_kwargs=None):
    x = np.ascontiguousarray(x, dtype=np.float32)
    labels = np.ascontiguousarray(labels).astype(np.int32)
    centers = np.ascontiguousarray(centers, dtype=np.float32)

    nc = get_nc()
    in_maps = [
        {
            "x": x[c * BPC:(c + 1) * BPC],
            "labels": labels[c * BPC:(c + 1) * BPC],
            "centers": centers,
        }
        for c in range(N_CORES)
    ]
    kwargs = _run_kwargs or {}
    out = run_bass_kernel_spmd(nc, in_maps, core_ids=list(range(N_CORES)),
                               **kwargs)
    # all-reduce the 8 per-core partial scalars (each already / BATCH)
    total = np.float32(0.0)
    for r in out.results:
        total = total + np.float32(r["out"][0, 0])
    if kwargs:
        kernel.last_run = out
    return np.asarray(total, dtype=np.float32)



# revision 17
# speedup vs baseline: 1.1158x; 1.1158x over previous
"""CenterLoss kernel for Trainium2 (raw Bass/Bacc, no Tile), 8-core
data-parallel.

Key algebraic insight: the reference builds the full [B, C] squared-
distance matrix and masks it with one-hot(labels), so only
distmat[i, labels[i]] survives.  The loss is therefore

    loss = (1/B) * sum_i || x_i - centers[labels[i]] ||^2
         = (1/B) * [ sum x^2  - 2 sum_i x_i . c_{l_i}  + sum_i ||c_{l_i}||^2 ]

which needs only a gather of each sample's center row, not the
[4096, 10000] matmul.

v2 design (vs v1's 4x indirect_dma_start + sub/square/PE chain):
  * Inputs staged in bf16 on host (x, centers): halves HBM/DMA traffic
    to ~1 MB per core.  Loss tolerance is 2e-2; bf16 error here is ~1e-3.
  * ONE indirect_dma_start with a [128, 4] offset AP (all 512 indices)
    pays the 994 ns SWDGE fixed cost once instead of 4x, and un-starves
    the gather DMA (descriptor generation no longer rate-limits it).
  * Expansion-form tail, one big fused op per engine, all running in
    parallel after the gather lands:
      Scalar : ACT Square accum over x  (early, during gather window)
               + ACT Square accum over gathered-c second half
      GpSimd : scalar_tensor_tensor accum over gathered-c first half
      Vector : tensor_tensor_reduce(x, c, mult, add, scale=-2) accum
    No PE / PSUM / final on-device reduction: the [128, 4] fp32 partial
    columns are DMA'd out per core and the host does the (blessed)
    all-reduce: loss = sum(all partials) / B.

Per core: 512 samples; sample s lives at (partition s%128, chunk s//128),
so the gather-offset tile is lab[p, c] = labels[c*128 + p] and the x tile
is loaded with the matching (c p) f -> p c f access pattern.

Manual semaphores; no Tile exit drain (bass entry preamble clears sems).
"""

from contextlib import ExitStack

import ml_dtypes
import numpy as np

import concourse.bacc as bacc
import concourse.bass as bass
from concourse import mybir

from concourse.bass_utils import run_bass_kernel_spmd

BATCH = 4096
NUM_CLASSES = 10000
FEAT_DIM = 512
N_CORES = 8
BPC = BATCH // N_CORES   # samples per core = 512
P = 128                  # SBUF partitions
CHUNKS = BPC // P        # 4 chunks of 128 samples per core
IDX_COLS = BPC // 16     # 32 int16 index columns (16-partition wrap)

AF = mybir.AluOpType
ACTF = mybir.ActivationFunctionType
BF16 = mybir.dt.bfloat16
_USE_BF16 = True
DT = BF16 if _USE_BF16 else mybir.dt.float32
NPDT_SRC = None

_NC_CACHE = {}
_ONE_GATHER = False
_USE_TTR = False
_USE_STT = True


def _build_bass():
    nc = bacc.Bacc(None, target_bir_lowering=False)

    x_in = nc.dram_tensor("x", [BPC, FEAT_DIM], DT, kind="ExternalInput")
    lab_in = nc.dram_tensor("labels", [P, CHUNKS], mybir.dt.int32,
                            kind="ExternalInput")
    cen_in = nc.dram_tensor("centers", [NUM_CLASSES, FEAT_DIM], DT,
                            kind="ExternalInput")
    out_t = nc.dram_tensor("out", [P, 4], mybir.dt.float32,
                           kind="ExternalOutput")

    with ExitStack() as ctx:
        ec = ctx.enter_context
        lab_sb = ec(nc.sbuf_tensor("lab_sb", [P, CHUNKS], mybir.dt.int32))
        xt = ec(nc.sbuf_tensor("xt", [P, CHUNKS * FEAT_DIM], DT))
        ct = ec(nc.sbuf_tensor("ct", [P, CHUNKS * FEAT_DIM], DT))
        # scratch for the mandatory elementwise outputs of the fused ops
        sv = ec(nc.sbuf_tensor("sv", [P, CHUNKS * FEAT_DIM], DT))
        sg = ec(nc.sbuf_tensor("sg", [P, 2 * FEAT_DIM], DT))
        ss = ec(nc.sbuf_tensor("ss", [P, CHUNKS * FEAT_DIM], DT))
        sc = ec(nc.sbuf_tensor("sc", [P, 2 * FEAT_DIM], DT))
        ss2 = ec(nc.sbuf_tensor("ss2", [P, CHUNKS * FEAT_DIM], DT))
        sc2 = ec(nc.sbuf_tensor("sc2", [P, 2 * FEAT_DIM], DT))
        accs = ec(nc.sbuf_tensor("accs", [P, 4], mybir.dt.float32))
        s_lab = ec(nc.semaphore("s_lab"))
        s_x = ec(nc.semaphore("s_x"))
        s_g = ec(nc.semaphore("s_g"))
        s_done = ec(nc.semaphore("s_done"))
        s_out = ec(nc.semaphore("s_out"))

        ct3 = ct[:].rearrange("p (c f) -> p c f", c=CHUNKS)

        # ---- Sync: index tile first (gather depends on it), then x as one
        # DMA (partition p <- rows {c*128+p}, 4 strips of 1 KB each).
        nc.sync.dma_start(out=lab_sb[:], in_=lab_in[:]).then_inc(s_lab, 16)
        nc.sync.dma_start(
            out=xt[:].rearrange("p (c f) -> p c f", c=CHUNKS),
            in_=x_in[:].rearrange("(c p) f -> p c f", p=P),
        ).then_inc(s_x, 16)

        # ---- GpSimd: single 512-row gather (SWDGE), then c^2 over the
        # first half of the gathered rows.
        G_TOTAL = 16 if _ONE_GATHER else 16 * CHUNKS
        nc.gpsimd.wait_ge(s_lab, 16)
        if _ONE_GATHER:
            nc.gpsimd.indirect_dma_start(
                out=ct3,
                out_offset=None,
                in_=cen_in[:],
                in_offset=bass.IndirectOffsetOnAxis(ap=lab_sb[:], axis=0),
            ).then_inc(s_g, 16)
        else:
            for a in range(CHUNKS):
                nc.gpsimd.indirect_dma_start(
                    out=ct[:, a * FEAT_DIM:(a + 1) * FEAT_DIM],
                    out_offset=None,
                    in_=cen_in[:],
                    in_offset=bass.IndirectOffsetOnAxis(
                        ap=lab_sb[:, a:a + 1], axis=0),
                ).then_inc(s_g, 16)
        # ---- Scalar: x^2 (early, inside the gather window), then c^2 over
        # the first half of the gathered rows.
        nc.scalar.wait_ge(s_x, 16)
        nc.scalar.activation(
            out=ss[:], in_=xt[:], func=ACTF.Square,
            accum_out=accs[:, 0:1]).then_inc(s_done, 1)
        nc.scalar.wait_ge(s_g, G_TOTAL)
        nc.scalar.activation(
            out=sc[:], in_=ct[:, :2 * FEAT_DIM], func=ACTF.Square,
            accum_out=accs[:, 1:2]).then_inc(s_done, 1)

        # ---- Vector: the cross term (one fused mult+reduce with the -2
        # folded into the instruction's scale), then c^2 over the second
        # half of the gathered rows.
        nc.vector.wait_ge(s_x, 16)
        nc.vector.wait_ge(s_g, G_TOTAL)
        if _USE_TTR:
            nc.vector.tensor_tensor_reduce(
                out=sv[:], in0=xt[:], in1=ct[:], scale=-2.0, scalar=0.0,
                op0=AF.mult, op1=AF.add,
                accum_out=accs[:, 3:4]).then_inc(s_done, 1)
            nc.vector.tensor_tensor_reduce(
                out=sg[:], in0=ct[:, 2 * FEAT_DIM:], in1=ct[:, 2 * FEAT_DIM:],
                scale=1.0, scalar=0.0, op0=AF.mult, op1=AF.add,
                accum_out=accs[:, 2:3]).then_inc(s_done, 1)
        elif _USE_STT:
            nc.vector.scalar_tensor_tensor(
                out=sv[:], in0=xt[:], scalar=-2.0, in1=ct[:],
                op0=AF.mult, op1=AF.mult,
                accum_out=accs[:, 3:4]).then_inc(s_done, 1)
            nc.vector.scalar_tensor_tensor(
                out=sg[:], in0=ct[:, 2 * FEAT_DIM:], scalar=1.0,
                in1=ct[:, 2 * FEAT_DIM:], op0=AF.mult, op1=AF.mult,
                accum_out=accs[:, 2:3]).then_inc(s_done, 1)
        else:
            s_vm = ec(nc.semaphore("s_vm"))
            nc.vector.tensor_tensor(
                out=sv[:], in0=xt[:], in1=ct[:],
                op=AF.mult).then_inc(s_vm, 1)
            nc.vector.tensor_tensor(
                out=sg[:], in0=ct[:, 2 * FEAT_DIM:], in1=ct[:, 2 * FEAT_DIM:],
                op=AF.mult).then_inc(s_vm, 1)
            nc.scalar.wait_ge(s_vm, 1)
            nc.scalar.activation(
                out=ss2[:], in_=sv[:], func=ACTF.Copy, scale=-2.0,
                accum_out=accs[:, 3:4]).then_inc(s_done, 1)
            nc.scalar.wait_ge(s_vm, 2)
            nc.scalar.activation(
                out=sc2[:], in_=sg[:], func=ACTF.Copy,
                accum_out=accs[:, 2:3]).then_inc(s_done, 1)

        # ---- Sync: output DMA of the four partial columns.  No completion
        # wait: the NRT exit barrier's per-engine Drain empties Sync's HWDGE
        # queue before execution is reported complete.
        nc.sync.wait_ge(s_done, 4)
        nc.sync.dma_start(out=out_t[:], in_=accs[:]).then_inc(s_out, 16)

    nc.compile()
    return nc


def get_nc():
    if "nc" not in _NC_CACHE:
        _NC_CACHE["nc"] = _build_bass()
    return _NC_CACHE["nc"]


def _idx_tile(labels_shard: np.ndarray) -> np.ndarray:
    """Gather-offset layout matching the (c p) f -> p c f x tile:
    lab[p, c] = labels[c*128 + p]."""
    return np.ascontiguousarray(labels_shard.astype(np.int32)
                                .reshape(CHUNKS, P).T)  # [128, 4]


def kernel(x, labels, centers, _run_kwargs=None):
    x = np.asarray(x, dtype=np.float32)
    if _USE_BF16:
        x = x.astype(ml_dtypes.bfloat16)
    labels = np.asarray(labels).astype(np.int64)
    centers = np.asarray(centers, dtype=np.float32)
    if _USE_BF16:
        centers = centers.astype(ml_dtypes.bfloat16)

    nc = get_nc()
    in_maps = [
        {
            "x": np.ascontiguousarray(x[c * BPC:(c + 1) * BPC]),
            "labels": _idx_tile(labels[c * BPC:(c + 1) * BPC]),
            "centers": centers,
        }
        for c in range(N_CORES)
    ]
    kwargs = _run_kwargs or {}
    out = run_bass_kernel_spmd(nc, in_maps, core_ids=list(range(N_CORES)),
                               **kwargs)
    # all-reduce the per-core partial-sum columns; mean over batch
    total = 0.0
    for r in out.results:
        total += float(r["out"].astype(np.float64).sum())
    if kwargs:
        kernel.last_run = out
    return np.asarray(total / BATCH, dtype=np.float32)


# revision 18
# speedup vs baseline: 1.2048x; 1.0798x over previous
"""CenterLoss kernel for Trainium2 (raw Bass/Bacc, no Tile), 8-core
data-parallel.

Key algebraic insight: the reference builds the full [B, C] squared-
distance matrix and masks it with one-hot(labels), so only
distmat[i, labels[i]] survives.  The loss is therefore

    loss = (1/B) * sum_i || x_i - centers[labels[i]] ||^2
         = (1/B) * [ sum x^2  - 2 sum_i x_i . c_{l_i}  + sum_i ||c_{l_i}||^2 ]

which needs only a gather of each sample's center row (indirect DMA),
not the [4096, 10000] matmul.

v3 design (vs v1's sub/square/PE chain):
  * Inputs staged in bf16 on host (x, centers): halves HBM/DMA traffic
    to ~1 MB per core.  Loss tolerance is 2e-2; measured bf16 error here
    is ~1e-5 (the three partial sums each average out quantization noise).
  * Expansion form, fused one-pass ops with free-dim accumulation:
      Vector : scalar_tensor_tensor (x*-2)*c, accum = -2 sum x.c
      Scalar : ACT Square with accum  = sum c^2
    per chunk, PIPELINED: each gather chunk is consumed the moment its
    DMA semaphore fires, so compute rides inside the gather window and
    only the last chunk's ~0.9 us remains on the tail.
  * sum x^2 runs before any gather lands (x arrives ~2.5 us earlier),
    split V/S so both engines are warm but free by chunk-0 time.
  * No PE / PSUM / on-device final reduce: the [128, 10] fp32 partial
    columns are DMA'd out per core and the host does the (blessed)
    all-reduce: loss = sum(all partials) / B.

The 4 indirect gathers stay 4 separate single-offset-column instructions:
SWDGE ucode only honors one offset per partition per instruction (a
[128, 4] offset AP writes ~1% of the destination -- probed on HW), so
128 rows x 4 is the minimum instruction count.

Per core: 512 samples; sample s lives at (partition s%128, chunk s//128),
so the gather-offset tile is lab[p, c] = labels[c*128 + p] and the x tile
is loaded with the matching (c p) f -> p c f access pattern.

Manual semaphores; no Tile exit drain (bass entry preamble clears sems).
"""

from contextlib import ExitStack

import ml_dtypes
import numpy as np

import concourse.bacc as bacc
import concourse.bass as bass
from concourse import mybir

from concourse.bass_utils import run_bass_kernel_spmd

BATCH = 4096
NUM_CLASSES = 10000
FEAT_DIM = 512
N_CORES = 8
BPC = BATCH // N_CORES   # samples per core = 512
P = 128                  # SBUF partitions
CHUNKS = BPC // P        # 4 chunks of 128 samples per core
HALF = CHUNKS * FEAT_DIM // 2
NCOL = 2 + 2 * CHUNKS    # accum columns: xsq_v, xsq_s, cc0-3, xc0-3

AF = mybir.AluOpType
ACTF = mybir.ActivationFunctionType
BF16 = mybir.dt.bfloat16

_NC_CACHE = {}


def _build_bass():
    nc = bacc.Bacc(None, target_bir_lowering=False)

    x_in = nc.dram_tensor("x", [BPC, FEAT_DIM], BF16, kind="ExternalInput")
    lab_in = nc.dram_tensor("labels", [P, CHUNKS], mybir.dt.int32,
                            kind="ExternalInput")
    cen_in = nc.dram_tensor("centers", [NUM_CLASSES, FEAT_DIM], BF16,
                            kind="ExternalInput")
    out_t = nc.dram_tensor("out", [P, NCOL], mybir.dt.float32,
                           kind="ExternalOutput")

    with ExitStack() as ctx:
        ec = ctx.enter_context
        lab_sb = ec(nc.sbuf_tensor("lab_sb", [P, CHUNKS], mybir.dt.int32))
        xt = ec(nc.sbuf_tensor("xt", [P, CHUNKS * FEAT_DIM], BF16))
        ct = ec(nc.sbuf_tensor("ct", [P, CHUNKS * FEAT_DIM], BF16))
        # scratch for the mandatory elementwise outputs of the fused ops
        sv = ec(nc.sbuf_tensor("sv", [P, CHUNKS * FEAT_DIM], BF16))
        ss = ec(nc.sbuf_tensor("ss", [P, CHUNKS * FEAT_DIM], BF16))
        accs = ec(nc.sbuf_tensor("accs", [P, NCOL], mybir.dt.float32))
        s_lab = ec(nc.semaphore("s_lab"))
        s_x = ec(nc.semaphore("s_x"))
        s_gs = [ec(nc.semaphore(f"s_g{a}")) for a in range(CHUNKS)]
        s_done = ec(nc.semaphore("s_done"))
        s_out = ec(nc.semaphore("s_out"))

        # ---- Sync: offset tile first (gathers depend on it), then x as
        # one DMA (partition p <- rows {c*128+p}, 4 strips of 1 KB each).
        nc.sync.dma_start(out=lab_sb[:], in_=lab_in[:]).then_inc(s_lab, 16)
        nc.sync.dma_start(
            out=xt[:].rearrange("p (c f) -> p c f", c=CHUNKS),
            in_=x_in[:].rearrange("(c p) f -> p c f", p=P),
        ).then_inc(s_x, 16)

        # ---- GpSimd: the four 128-row gathers (SWDGE, ~1.1 us each).
        nc.gpsimd.wait_ge(s_lab, 16)
        for a in range(CHUNKS):
            nc.gpsimd.indirect_dma_start(
                out=ct[:, a * FEAT_DIM:(a + 1) * FEAT_DIM],
                out_offset=None,
                in_=cen_in[:],
                in_offset=bass.IndirectOffsetOnAxis(
                    ap=lab_sb[:, a:a + 1], axis=0),
            ).then_inc(s_gs[a], 16)

        # ---- Vector: half of sum x^2 while gathers run, then per-chunk
        # -2 sum x.c the moment each gather chunk lands.
        nc.vector.wait_ge(s_x, 16)
        nc.vector.scalar_tensor_tensor(
            out=sv[:, :HALF], in0=xt[:, :HALF], scalar=1.0,
            in1=xt[:, :HALF], op0=AF.mult, op1=AF.mult,
            accum_out=accs[:, 0:1]).then_inc(s_done, 1)
        for a in range(CHUNKS):
            sl = slice(a * FEAT_DIM, (a + 1) * FEAT_DIM)
            nc.vector.wait_ge(s_gs[a], 16)
            nc.vector.scalar_tensor_tensor(
                out=sv[:, sl], in0=xt[:, sl], scalar=-2.0, in1=ct[:, sl],
                op0=AF.mult, op1=AF.mult,
                accum_out=accs[:, 2 + CHUNKS + a:3 + CHUNKS + a],
            ).then_inc(s_done, 1)

        # ---- Scalar: other half of sum x^2, then per-chunk sum c^2.
        nc.scalar.wait_ge(s_x, 16)
        nc.scalar.activation(
            out=ss[:, :HALF], in_=xt[:, HALF:], func=ACTF.Square,
            accum_out=accs[:, 1:2]).then_inc(s_done, 1)
        for a in range(CHUNKS):
            sl = slice(a * FEAT_DIM, (a + 1) * FEAT_DIM)
            nc.scalar.wait_ge(s_gs[a], 16)
            nc.scalar.activation(
                out=ss[:, sl], in_=ct[:, sl], func=ACTF.Square,
                accum_out=accs[:, 2 + a:3 + a]).then_inc(s_done, 1)

        # ---- Sync: output DMA of the partial columns.  No completion
        # wait: the NRT exit barrier's per-engine Drain empties Sync's
        # HWDGE queue before execution is reported complete.
        nc.sync.wait_ge(s_done, NCOL)
        nc.sync.dma_start(out=out_t[:], in_=accs[:]).then_inc(s_out, 16)

    nc.compile()
    return nc


def get_nc():
    if "nc" not in _NC_CACHE:
        _NC_CACHE["nc"] = _build_bass()
    return _NC_CACHE["nc"]


def _idx_tile(labels_shard: np.ndarray) -> np.ndarray:
    """Gather-offset layout matching the (c p) f -> p c f x tile:
    lab[p, c] = labels[c*128 + p]."""
    return np.ascontiguousarray(labels_shard.astype(np.int32)
                                .reshape(CHUNKS, P).T)  # [128, 4]


def kernel(x, labels, centers, _run_kwargs=None):
    x = np.asarray(x, dtype=np.float32).astype(ml_dtypes.bfloat16)
    labels = np.asarray(labels).astype(np.int64)
    centers = np.asarray(centers, dtype=np.float32).astype(ml_dtypes.bfloat16)

    nc = get_nc()
    in_maps = [
        {
            "x": np.ascontiguousarray(x[c * BPC:(c + 1) * BPC]),
            "labels": _idx_tile(labels[c * BPC:(c + 1) * BPC]),
            "centers": centers,
        }
        for c in range(N_CORES)
    ]
    kwargs = _run_kwargs or {}
    out = run_bass_kernel_spmd(nc, in_maps, core_ids=list(range(N_CORES)),
                               **kwargs)
    # all-reduce the per-core partial-sum columns; mean over batch
    total = 0.0
    for r in out.results:
        total += float(r["out"].astype(np.float64).sum())
    if kwargs:
        kernel.last_run = out
    return np.asarray(total / BATCH, dtype=np.float32)
